# revision 1
# baseline (speedup 1.0000x reference)
"""Trainium2 Bass kernel for ContrastiveGNN (3x GCNConv + 2-layer projector).

Sharding: nodes are partitioned across 8 NeuronCores. Host relabels nodes by
in-degree into bands of 1024 (8 cores x 128 partitions) so every core's
tile g has the same max in-degree K[g] -> identical SPMD instruction stream
with ~1% gather padding. Per layer: local matmul (h = act @ W), fold
deg_rsqrt into h (p' = h * r), AllGather p' into a full table, then each core
gathers its padded in-edge sources (one 128-row indirect DMA per slot) and
reduces along the slot axis. agg = r * sum(p'[src]) + h * deg_inv + b.
"""
import numpy as np

import concourse.bacc as bacc
import concourse.bass as bass
import concourse.mybir as mybir
import concourse.tile as tile
from concourse.masks import make_identity

N = 100000
E = 1600000
IN_D, HID, OUT_D = 256, 64, 64
N_CORES = 8
P = 128
F32 = mybir.dt.float32
BF16 = mybir.dt.bfloat16


def _plan(edge_index: np.ndarray, n: int):
    band = N_CORES * P
    n_bands = (n + band - 1) // band
    rpc = n_bands * P
    ntot = N_CORES * rpc

    src = np.asarray(edge_index[0], dtype=np.int64)
    dst = np.asarray(edge_index[1], dtype=np.int64)
    ne = len(src)
    deg = np.bincount(dst, minlength=n).astype(np.float64) + 1.0

    order = np.argsort(-deg, kind="stable")
    i = np.arange(n)
    g, s = i // band, i % band
    c, p = s // P, s % P
    new_id = np.empty(n, dtype=np.int64)
    new_id[order] = c * rpc + g * P + p

    nsrc = new_id[src]
    ndst = new_id[dst]

    cnt = np.bincount(ndst, minlength=ntot)
    K = cnt.reshape(N_CORES, n_bands, P).max(axis=(0, 2)).astype(np.int64)
    K = np.maximum(K, 1)
    CUM = np.concatenate([[0], np.cumsum(K)])
    SLOTS = int(CUM[-1])

    # pad target: dummy row (x=0, dr=0 -> p'=0 every layer); needs n < ntot
    pad_row = ntot - 1
    assert n < ntot, "need at least one dummy node for the zero row"

    eo = np.argsort(ndst, kind="stable")
    sdst, ssrc = ndst[eo], nsrc[eo]
    first = np.searchsorted(sdst, sdst, side="left")
    kidx = np.arange(ne) - first
    ec = sdst // rpc
    loc = sdst % rpc
    eg = loc // P
    ep = loc % P
    col = CUM[eg] + kidx
    sidx = np.full((N_CORES, P, SLOTS), pad_row, dtype=np.int32)
    sidx[ec, ep, col] = ssrc.astype(np.int32)

    dr = (1.0 / np.sqrt(deg)).astype(np.float32)
    di = (1.0 / deg).astype(np.float32)
    drn = np.zeros(ntot, np.float32)
    din = np.zeros(ntot, np.float32)
    drn[new_id] = dr
    din[new_id] = di
    degc = np.zeros((N_CORES, P, 2 * n_bands), np.float32)
    degc[:, :, :n_bands] = drn.reshape(N_CORES, n_bands, P).transpose(0, 2, 1)
    degc[:, :, n_bands:] = din.reshape(N_CORES, n_bands, P).transpose(0, 2, 1)

    return dict(new_id=new_id, K=[int(k) for k in K], CUM=[int(x) for x in CUM],
                SLOTS=SLOTS, sidx=sidx, degc=degc, pad_row=pad_row,
                n_bands=n_bands, rpc=rpc, ntot=ntot, n=n)


def _build(pl, in_d=IN_D, no_gather=False, no_coll=False, no_reduce=False,
           table_bf16=False):
    K, CUM, SLOTS = pl["K"], pl["CUM"], pl["SLOTS"]
    n_bands, rpc, ntot = pl["n_bands"], pl["rpc"], pl["ntot"]

    nc = bacc.Bacc("TRN2", target_bir_lowering=False, debug=False,
                   num_devices=N_CORES)
    xT = nc.dram_tensor("xT", [in_d, rpc], F32, kind="ExternalInput")
    W1 = nc.dram_tensor("W1", [in_d, HID], F32, kind="ExternalInput")
    W2 = nc.dram_tensor("W2", [HID, HID], F32, kind="ExternalInput")
    W3 = nc.dram_tensor("W3", [HID, OUT_D], F32, kind="ExternalInput")
    Pw1 = nc.dram_tensor("Pw1", [OUT_D, HID], F32, kind="ExternalInput")
    Pw2 = nc.dram_tensor("Pw2", [HID, OUT_D], F32, kind="ExternalInput")
    brep = nc.dram_tensor("brep", [P, 5 * 64], F32, kind="ExternalInput")
    degc = nc.dram_tensor("degc", [P, 2 * n_bands], F32, kind="ExternalInput")
    sidx = nc.dram_tensor("sidx", [P, SLOTS], mybir.dt.int32,
                          kind="ExternalInput")
    z = nc.dram_tensor("z", [rpc, 64], F32, kind="ExternalOutput")

    kchunks = (in_d + P - 1) // P
    with tile.TileContext(nc) as tc:
        with (
            tc.tile_pool(name="const", bufs=1) as cpool,
            tc.tile_pool(name="acts", bufs=1) as apool,
            tc.tile_pool(name="work", bufs=3) as wpool,
            tc.tile_pool(name="gbuf", bufs=4) as gpool,
            tc.tile_pool(name="psmm", bufs=4, space="PSUM") as psmm,
            tc.tile_pool(name="pstr", bufs=4, space="PSUM") as pstr,
            tc.tile_pool(name="dram", bufs=2, space="DRAM") as dpool,
        ):
            xTs = []
            w1s = []
            for kc in range(kchunks):
                kp = min(P, in_d - kc * P)
                xc = cpool.tile([kp, rpc], F32, tag=f"xT{kc}")
                nc.sync.dma_start(xc[:], xT[kc * P:kc * P + kp, :])
                xTs.append(xc)
                wc = cpool.tile([kp, HID], F32, tag=f"w1_{kc}")
                nc.sync.dma_start(wc[:], W1[kc * P:kc * P + kp, :])
                w1s.append(wc)
            w2 = cpool.tile([HID, HID], F32, tag="w2")
            w3 = cpool.tile([HID, OUT_D], F32, tag="w3")
            pw1 = cpool.tile([OUT_D, HID], F32, tag="pw1")
            pw2 = cpool.tile([HID, OUT_D], F32, tag="pw2")
            nc.sync.dma_start(w2[:], W2[:])
            nc.sync.dma_start(w3[:], W3[:])
            nc.sync.dma_start(pw1[:], Pw1[:])
            nc.sync.dma_start(pw2[:], Pw2[:])
            bsb = cpool.tile([P, 5 * 64], F32, tag="bsb")
            dsb = cpool.tile([P, 2 * n_bands], F32, tag="dsb")
            isb = cpool.tile([P, SLOTS], mybir.dt.int32, tag="isb")
            nc.sync.dma_start(bsb[:], brep[:])
            nc.sync.dma_start(dsb[:], degc[:])
            nc.sync.dma_start(isb[:], sidx[:])
            ident = cpool.tile([P, P], F32, tag="ident")
            make_identity(nc, ident[:])

            p_sb = apool.tile([P, n_bands * 64], F32, tag="p_sb")
            actA = apool.tile([P, n_bands * 64], F32, tag="actA")
            GRP = 14
            n_grp = (n_bands + GRP - 1) // GRP

            act = None
            TDT = BF16 if table_bf16 else F32
            for L in range(3):
                ag_in = dpool.tile([rpc, 64], TDT, tag="ag_in")
                table = dpool.tile([ntot, 64], TDT, tag="table",
                                   addr_space="Shared")
                wl = [None, w2, w3][L]
                for t in range(n_bands):
                    tb = slice(t * 64, (t + 1) * 64)
                    ps_h = psmm.tile([P, 64], F32, tag="ps_h")
                    if L == 0:
                        for kc in range(kchunks):
                            nc.tensor.matmul(ps_h[:],
                                             xTs[kc][:, t * P:(t + 1) * P],
                                             w1s[kc][:],
                                             start=(kc == 0),
                                             stop=(kc == kchunks - 1))
                    else:
                        ps_tr = pstr.tile([64, P], F32, tag="ps_tr")
                        nc.tensor.transpose(ps_tr[:], act[:, tb], ident[:])
                        lh = wpool.tile([64, P], F32, tag="lh")
                        nc.vector.tensor_copy(lh[:], ps_tr[:])
                        nc.tensor.matmul(ps_h[:], lh[:], wl[:],
                                         start=True, stop=True)
                    nc.vector.tensor_scalar_mul(p_sb[:, tb], ps_h[:],
                                                dsb[:, t:t + 1])
                    if t % GRP == GRP - 1 or t == n_bands - 1:
                        g0 = (t // GRP) * GRP
                        ng = t - g0 + 1
                        nc.sync.dma_start(
                            ag_in[g0 * P:(t + 1) * P, :].rearrange(
                                "(tt p) f -> p tt f", p=P),
                            p_sb[:, g0 * 64:(t + 1) * 64].rearrange(
                                "p (tt f) -> p tt f", f=64))

                if not no_coll:
                    nc.gpsimd.collective_compute(
                        "AllGather",
                        mybir.AluOpType.bypass,
                        replica_groups=[list(range(N_CORES))],
                        ins=[ag_in.opt()],
                        outs=[table.opt()],
                    )

                for t in range(n_bands):
                    tb = slice(t * 64, (t + 1) * 64)
                    Kt = K[t]
                    c0 = CUM[t]
                    acc = wpool.tile([P, 64], F32, tag="acc")
                    if no_gather:
                        nc.vector.memset(acc[:], 0.0)
                    else:
                        gt = gpool.tile([P, Kt, 64], TDT, tag="g")
                        for k in range(Kt):
                            nc.gpsimd.indirect_dma_start(
                                out=gt[:, k], out_offset=None, in_=table[:],
                                in_offset=bass.IndirectOffsetOnAxis(
                                    ap=isb[:, c0 + k:c0 + k + 1], axis=0),
                            )
                        if no_reduce:
                            nc.vector.memset(acc[:], 0.0)
                        else:
                            nc.vector.reduce_sum(
                                out=acc[:],
                                in_=gt[:].rearrange("p k f -> p f k"),
                                axis=mybir.AxisListType.X)
                    t1 = wpool.tile([P, 64], F32, tag="t1")
                    nc.vector.tensor_add(t1[:], acc[:], p_sb[:, tb])
                    nc.vector.tensor_scalar_mul(t1[:], t1[:], dsb[:, t:t + 1])
                    nc.vector.tensor_add(t1[:], t1[:],
                                         bsb[:, L * 64:(L + 1) * 64])
                    nc.scalar.activation(actA[:, tb], t1[:],
                                         mybir.ActivationFunctionType.Relu)
                act = actA

            for t in range(n_bands):
                tb = slice(t * 64, (t + 1) * 64)
                ps_tr = pstr.tile([64, P], F32, tag="ps_tr")
                nc.tensor.transpose(ps_tr[:], act[:, tb], ident[:])
                lh = wpool.tile([64, P], F32, tag="lh")
                nc.vector.tensor_copy(lh[:], ps_tr[:])
                ps_q = psmm.tile([P, 64], F32, tag="ps_h")
                nc.tensor.matmul(ps_q[:], lh[:], pw1[:], start=True, stop=True)
                q0 = wpool.tile([P, 64], F32, tag="q0")
                nc.vector.tensor_add(q0[:], ps_q[:], bsb[:, 3 * 64:4 * 64])
                q = wpool.tile([P, 64], F32, tag="q")
                nc.scalar.activation(q[:], q0[:],
                                     mybir.ActivationFunctionType.Relu)
                ps_tr2 = pstr.tile([64, P], F32, tag="ps_tr")
                nc.tensor.transpose(ps_tr2[:], q[:], ident[:])
                lh2 = wpool.tile([64, P], F32, tag="lh")
                nc.vector.tensor_copy(lh2[:], ps_tr2[:])
                ps_z = psmm.tile([P, 64], F32, tag="ps_h")
                nc.tensor.matmul(ps_z[:], lh2[:], pw2[:], start=True, stop=True)
                nc.vector.tensor_add(p_sb[:, tb], ps_z[:],
                                     bsb[:, 4 * 64:5 * 64])
                if t % GRP == GRP - 1 or t == n_bands - 1:
                    g0 = (t // GRP) * GRP
                    nc.sync.dma_start(
                        z[g0 * P:(t + 1) * P, :].rearrange(
                            "(tt p) f -> p tt f", p=P),
                        p_sb[:, g0 * 64:(t + 1) * 64].rearrange(
                            "p (tt f) -> p tt f", f=64))

    nc.compile()
    return nc


def _in_maps(inputs, pl, in_d=IN_D):
    x = np.asarray(inputs["x"], np.float32)
    new_id = pl["new_id"]
    rpc, ntot = pl["rpc"], pl["ntot"]
    xn = np.zeros((ntot, in_d), np.float32)
    xn[new_id] = x
    brep = np.tile(
        np.concatenate([
            np.asarray(inputs["b1"], np.float32),
            np.asarray(inputs["b2"], np.float32),
            np.asarray(inputs["b3"], np.float32),
            np.asarray(inputs["Pb1"], np.float32),
            np.asarray(inputs["Pb2"], np.float32),
        ])[None, :], (P, 1))
    common = dict(
        W1=np.asarray(inputs["W1"], np.float32),
        W2=np.asarray(inputs["W2"], np.float32),
        W3=np.asarray(inputs["W3"], np.float32),
        Pw1=np.asarray(inputs["Pw1"], np.float32),
        Pw2=np.asarray(inputs["Pw2"], np.float32),
        brep=brep,
    )
    maps = []
    for c in range(N_CORES):
        xc = xn[c * rpc:(c + 1) * rpc]
        maps.append(dict(
            xT=np.ascontiguousarray(xc.T),
            degc=pl["degc"][c],
            sidx=pl["sidx"][c],
            **common,
        ))
    return maps


def build_all(inputs, n=None, in_d=IN_D, **bkw):
    x = np.asarray(inputs["x"])
    n = x.shape[0] if n is None else n
    pl = _plan(np.asarray(inputs["edge_index"]), n)
    nc = _build(pl, in_d=in_d, **bkw)
    maps = _in_maps(inputs, pl, in_d=in_d)
    return nc, maps, pl


def postprocess(results, pl):
    z_new = np.concatenate([results[c]["z"] for c in range(N_CORES)], axis=0)
    return np.ascontiguousarray(z_new[pl["new_id"]]).astype(np.float32)


def kernel(**inputs) -> np.ndarray:
    from concourse.bass_utils import run_bass_kernel_spmd
    nc, maps, pl = build_all(inputs)
    res = run_bass_kernel_spmd(nc, maps, core_ids=list(range(N_CORES)))
    return postprocess(res.results, pl)

